# revision 1
# baseline (speedup 1.0000x reference)
"""PeriodicDistance on 8 NeuronCores: edge-sharded gather + periodic vector math.

Per core: 800k edges laid out [128 partitions x 6250 cols]. pos (200k x 3) and
box replicated. Gathers via SWDGE indirect DMA (128 rows/inst, one inst per
edge column per endpoint); DVE computes vec = pos[i0] - pos[i1] - shifts@box,
ACT computes sqrt of the squared norm.
"""
import numpy as np

N_ATOMS = 200_000
N_EDGES = 6_400_000
N_CORES = 8
EC = N_EDGES // N_CORES      # 800_000 edges per core
P = 128
C = EC // P                  # 6250 edge columns per partition
K = 625                      # columns per tile
NT = C // K                  # 10 tiles
D = 3

_CACHE = {}


def _build():
    import concourse.bass as bass
    import concourse.tile as tile
    from concourse import bacc, mybir

    nc = bacc.Bacc()
    pos = nc.dram_tensor("pos", [N_ATOMS, D], mybir.dt.float32, kind="ExternalInput")
    nbox = nc.dram_tensor("nbox", [1, 9], mybir.dt.float32, kind="ExternalInput")
    idx0 = nc.dram_tensor("idx0", [P, C], mybir.dt.int32, kind="ExternalInput")
    idx1 = nc.dram_tensor("idx1", [P, C], mybir.dt.int32, kind="ExternalInput")
    sh = nc.dram_tensor("sh", [P, C, D], mybir.dt.float32, kind="ExternalInput")
    outv = nc.dram_tensor("outv", [P, C, D], mybir.dt.float32, kind="ExternalOutput")
    outw = nc.dram_tensor("outw", [P, C], mybir.dt.float32, kind="ExternalOutput")

    with tile.TileContext(nc) as tc:
        with tc.tile_pool(name="const", bufs=1) as cpool, \
             tc.tile_pool(name="idx", bufs=2) as ipool, \
             tc.tile_pool(name="g", bufs=2) as gpool, \
             tc.tile_pool(name="cmp", bufs=2) as vpool:
            nb_t = cpool.tile([P, 9], mybir.dt.float32)
            nc.sync.dma_start(nb_t[:], nbox[:].to_broadcast((P, 9)))
            for t in range(NT):
                c0 = t * K
                i0_t = ipool.tile([P, K], mybir.dt.int32)
                i1_t = ipool.tile([P, K], mybir.dt.int32)
                s_t = ipool.tile([P, K, D], mybir.dt.float32)
                nc.sync.dma_start(i0_t[:], idx0[:, c0:c0 + K])
                nc.sync.dma_start(i1_t[:], idx1[:, c0:c0 + K])
                nc.sync.dma_start(s_t[:], sh[:, c0:c0 + K, :])
                g0_t = gpool.tile([P, K, D], mybir.dt.float32)
                g1_t = gpool.tile([P, K, D], mybir.dt.float32)
                for j in range(K):
                    nc.gpsimd.indirect_dma_start(
                        out=g0_t[:, j, :], out_offset=None, in_=pos[:],
                        in_offset=bass.IndirectOffsetOnAxis(ap=i0_t[:, j:j + 1], axis=0),
                    )
                    nc.gpsimd.indirect_dma_start(
                        out=g1_t[:, j, :], out_offset=None, in_=pos[:],
                        in_offset=bass.IndirectOffsetOnAxis(ap=i1_t[:, j:j + 1], axis=0),
                    )
                vec_t = vpool.tile([P, K, D], mybir.dt.float32)
                sq_t = vpool.tile([P, K, D], mybir.dt.float32)
                w_t = vpool.tile([P, K], mybir.dt.float32)
                nc.vector.tensor_tensor(
                    out=vec_t[:, :, :], in0=g0_t[:, :, :], in1=g1_t[:, :, :],
                    op=mybir.AluOpType.subtract,
                )
                # vec[:,:,m] += shifts[:,:,n] * (-box[n,m])
                for m in range(3):
                    for n in range(3):
                        nc.vector.scalar_tensor_tensor(
                            out=vec_t[:, :, m], in0=s_t[:, :, n],
                            scalar=nb_t[:, 3 * n + m:3 * n + m + 1],
                            in1=vec_t[:, :, m],
                            op0=mybir.AluOpType.mult, op1=mybir.AluOpType.add,
                        )
                nc.vector.tensor_tensor(
                    out=sq_t[:, :, :], in0=vec_t[:, :, :], in1=vec_t[:, :, :],
                    op=mybir.AluOpType.mult,
                )
                nc.vector.tensor_reduce(
                    w_t[:], sq_t[:, :, :], mybir.AxisListType.X, mybir.AluOpType.add,
                )
                nc.scalar.activation(
                    out=w_t[:], in_=w_t[:], func=mybir.ActivationFunctionType.Sqrt,
                )
                nc.sync.dma_start(outv[:, c0:c0 + K, :], vec_t[:, :, :])
                nc.sync.dma_start(outw[:, c0:c0 + K], w_t[:])

    nc.compile()
    return nc


def kernel(pos, box, edge_index, shifts_idx, batch_map=None):
    from concourse.bass_utils import run_bass_kernel_spmd

    pos = np.ascontiguousarray(np.asarray(pos, dtype=np.float32))
    box = np.asarray(box, dtype=np.float32)
    edge_index = np.asarray(edge_index, dtype=np.int32)
    shifts_idx = np.asarray(shifts_idx, dtype=np.int32)

    if "nc" not in _CACHE:
        _CACHE["nc"] = _build()
    nc = _CACHE["nc"]

    nbox9 = np.ascontiguousarray((-box[0]).reshape(1, 9))
    i0 = edge_index[0].reshape(N_CORES, P, C)
    i1 = edge_index[1].reshape(N_CORES, P, C)
    shf = shifts_idx.astype(np.float32).reshape(N_CORES, P, C, D)
    in_maps = [
        {
            "pos": pos,
            "nbox": nbox9,
            "idx0": np.ascontiguousarray(i0[c]),
            "idx1": np.ascontiguousarray(i1[c]),
            "sh": np.ascontiguousarray(shf[c]),
        }
        for c in range(N_CORES)
    ]
    res = run_bass_kernel_spmd(nc, in_maps, core_ids=list(range(N_CORES)))
    edge_vec = np.concatenate(
        [res.results[c]["outv"].reshape(EC, D) for c in range(N_CORES)], axis=0
    )
    edge_weight = np.concatenate(
        [res.results[c]["outw"].reshape(EC) for c in range(N_CORES)], axis=0
    )
    return edge_index, edge_weight, edge_vec, shifts_idx


# revision 2
# speedup vs baseline: 217.9036x; 217.9036x over previous
"""PeriodicDistance on 8 NeuronCores: edge-sharded gather + periodic vector math.

Per core: 800k edges laid out [128 partitions x 6250 cols]. pos (200k x 3) and
box replicated. Gathers via SWDGE indirect DMA (128 rows/inst, one inst per
edge column per endpoint); DVE computes vec = (pos[i0]-pos[i1]) - shifts@box
in the reference's rounding order (bit-exact), ACT sqrt + one DVE Newton step
for the edge weights.
"""
import numpy as np

N_ATOMS = 200_000
N_EDGES = 6_400_000
N_CORES = 8
EC = N_EDGES // N_CORES      # 800_000 edges per core
P = 128
C = EC // P                  # 6250 edge columns per partition
K = 625                      # columns per tile
NT = C // K                  # 10 tiles
D = 3

_CACHE = {}


def _build():
    import concourse.bass as bass
    import concourse.tile as tile
    from concourse import bacc, mybir

    nc = bacc.Bacc()
    pos = nc.dram_tensor("pos", [N_ATOMS, D], mybir.dt.float32, kind="ExternalInput")
    boxm = nc.dram_tensor("boxm", [1, 9], mybir.dt.float32, kind="ExternalInput")
    idx0 = nc.dram_tensor("idx0", [P, C], mybir.dt.int32, kind="ExternalInput")
    idx1 = nc.dram_tensor("idx1", [P, C], mybir.dt.int32, kind="ExternalInput")
    sh8 = nc.dram_tensor("sh8", [P, C, D], mybir.dt.int8, kind="ExternalInput")
    outv = nc.dram_tensor("outv", [P, C, D], mybir.dt.float32, kind="ExternalOutput")
    outw = nc.dram_tensor("outw", [P, C], mybir.dt.float32, kind="ExternalOutput")

    with tile.TileContext(nc) as tc:
        with tc.tile_pool(name="const", bufs=1) as cpool, \
             tc.tile_pool(name="idx", bufs=2) as ipool, \
             tc.tile_pool(name="g", bufs=2) as gpool, \
             tc.tile_pool(name="cmp", bufs=2) as vpool:
            b_t = cpool.tile([P, 9], mybir.dt.float32)
            nc.sync.dma_start(b_t[:], boxm[:].to_broadcast((P, 9)))
            for t in range(NT):
                c0 = t * K
                i0_t = ipool.tile([P, K], mybir.dt.int32)
                i1_t = ipool.tile([P, K], mybir.dt.int32)
                s8_t = ipool.tile([P, K, D], mybir.dt.int8)
                nc.sync.dma_start(i0_t[:], idx0[:, c0:c0 + K])
                nc.sync.dma_start(i1_t[:], idx1[:, c0:c0 + K])
                nc.sync.dma_start(s8_t[:], sh8[:, c0:c0 + K, :])
                g0_t = gpool.tile([P, K, D], mybir.dt.float32)
                g1_t = gpool.tile([P, K, D], mybir.dt.float32)
                for j in range(K):
                    nc.gpsimd.indirect_dma_start(
                        out=g0_t[:, j, :], out_offset=None, in_=pos[:],
                        in_offset=bass.IndirectOffsetOnAxis(ap=i0_t[:, j:j + 1], axis=0),
                    )
                    nc.gpsimd.indirect_dma_start(
                        out=g1_t[:, j, :], out_offset=None, in_=pos[:],
                        in_offset=bass.IndirectOffsetOnAxis(ap=i1_t[:, j:j + 1], axis=0),
                    )
                s_t = vpool.tile([P, K, D], mybir.dt.float32)
                cs_t = vpool.tile([P, K, D], mybir.dt.float32)
                vec_t = vpool.tile([P, K, D], mybir.dt.float32)
                sq_t = vpool.tile([P, K, D], mybir.dt.float32)
                x_t = vpool.tile([P, K], mybir.dt.float32)
                w_t = vpool.tile([P, K], mybir.dt.float32)
                r_t = vpool.tile([P, K], mybir.dt.float32)
                nc.vector.tensor_copy(out=s_t[:, :, :], in_=s8_t[:, :, :])
                nc.vector.tensor_tensor(
                    out=vec_t[:, :, :], in0=g0_t[:, :, :], in1=g1_t[:, :, :],
                    op=mybir.AluOpType.subtract,
                )
                # cs[:,:,m] = ((s0*b[0,m] + s1*b[1,m]) + s2*b[2,m]) -- einsum order
                for m in range(3):
                    nc.vector.tensor_scalar(
                        out=cs_t[:, :, m], in0=s_t[:, :, 0],
                        scalar1=b_t[:, m:m + 1], scalar2=None,
                        op0=mybir.AluOpType.mult,
                    )
                    for n in (1, 2):
                        nc.vector.scalar_tensor_tensor(
                            out=cs_t[:, :, m], in0=s_t[:, :, n],
                            scalar=b_t[:, 3 * n + m:3 * n + m + 1], in1=cs_t[:, :, m],
                            op0=mybir.AluOpType.mult, op1=mybir.AluOpType.add,
                        )
                nc.vector.tensor_tensor(
                    out=vec_t[:, :, :], in0=vec_t[:, :, :], in1=cs_t[:, :, :],
                    op=mybir.AluOpType.subtract,
                )
                nc.vector.tensor_tensor(
                    out=sq_t[:, :, :], in0=vec_t[:, :, :], in1=vec_t[:, :, :],
                    op=mybir.AluOpType.mult,
                )
                nc.vector.tensor_reduce(
                    x_t[:], sq_t[:, :, :], mybir.AxisListType.X, mybir.AluOpType.add,
                )
                nc.scalar.activation(
                    out=w_t[:], in_=x_t[:], func=mybir.ActivationFunctionType.Sqrt,
                )
                # Newton: w = 0.5*(w + x*recip(max(w,1e-20)))
                nc.vector.tensor_scalar_max(out=r_t[:], in0=w_t[:], scalar1=1e-20)
                nc.vector.reciprocal(out=r_t[:], in_=r_t[:])
                nc.vector.tensor_tensor(out=r_t[:], in0=x_t[:], in1=r_t[:],
                                        op=mybir.AluOpType.mult)
                nc.vector.tensor_tensor(out=r_t[:], in0=w_t[:], in1=r_t[:],
                                        op=mybir.AluOpType.add)
                nc.vector.tensor_scalar(out=w_t[:], in0=r_t[:], scalar1=0.5,
                                        scalar2=None, op0=mybir.AluOpType.mult)
                nc.sync.dma_start(outv[:, c0:c0 + K, :], vec_t[:, :, :])
                nc.sync.dma_start(outw[:, c0:c0 + K], w_t[:])

    nc.compile()
    return nc


def kernel(pos, box, edge_index, shifts_idx, batch_map=None):
    from concourse.bass_utils import run_bass_kernel_spmd

    pos = np.ascontiguousarray(np.asarray(pos, dtype=np.float32))
    box = np.asarray(box, dtype=np.float32)
    edge_index = np.asarray(edge_index, dtype=np.int32)
    shifts_idx = np.asarray(shifts_idx, dtype=np.int32)

    if "nc" not in _CACHE:
        _CACHE["nc"] = _build()
    nc = _CACHE["nc"]

    box9 = np.ascontiguousarray(box[0].reshape(1, 9))
    i0 = edge_index[0].reshape(N_CORES, P, C)
    i1 = edge_index[1].reshape(N_CORES, P, C)
    sh = shifts_idx.astype(np.int8).reshape(N_CORES, P, C, D)
    in_maps = [
        {
            "pos": pos,
            "boxm": box9,
            "idx0": np.ascontiguousarray(i0[c]),
            "idx1": np.ascontiguousarray(i1[c]),
            "sh8": np.ascontiguousarray(sh[c]),
        }
        for c in range(N_CORES)
    ]
    res = run_bass_kernel_spmd(nc, in_maps, core_ids=list(range(N_CORES)))
    edge_vec = np.concatenate(
        [res.results[c]["outv"].reshape(EC, D) for c in range(N_CORES)], axis=0
    )
    edge_weight = np.concatenate(
        [res.results[c]["outw"].reshape(EC) for c in range(N_CORES)], axis=0
    )
    return edge_index, edge_weight, edge_vec, shifts_idx
